# revision 52
# baseline (speedup 1.0000x reference)
"""Grouped-Query Attention (B=2, S=2048, d_model=1024, 16 Q heads / 4 KV heads)
as a Trainium2 Bass/Tile kernel over 8 NeuronCores.

Sharding: core c = (batch b, kv-group g) with b = c//4, g = c%4.
Each core owns 4 consecutive Q heads (4g..4g+3) and their shared KV head g:
  - W_q column-shard [1024, 256], W_k/W_v column-shard [1024, 64],
  - W_o row-shard [256, 1024]  -> per-core partial output, host sums the
    4 group partials per batch.

Device dataflow (all matmuls contract over the SBUF partition dim):
  inputs are host-transposed to x^T [d_model, S]; projections run with the
  weight as the stationary operand:
    K^T[64,S]   = Wk^T  @ k^T   (stored twice on partitions 0-63/64-127)
    Q^T[256,S]  = Wq^T  @ q^T   (two tiles of 128 = head pairs)
    V[S,65]     = v @ Wv, plus a ones column (rowsum trick)
  attention as a 3-deep software pipeline over steps s = (q-chunk, head
  pair), with the stages' PE work INTERLEAVED in emission order so the PE
  never waits on the exp engine or the DVE reciprocal chain:
    stage 1 (step s):   S^T = K_blk @ Q^T ; P^T = exp(S^T/8)  (scalar eng)
    stage 2 (step s-1): O^T[65,q] = V_aug^T @ P^T ; copy PSUM->SBUF bf16;
                        reciprocal of denominator row (DVE)
    stage 3 (step s-2): one K=2 selector-matmul broadcasts both heads'
                        reciprocals to their partition halves; O_norm = O*r;
                        after odd steps, the head-paired output projection
"""

import sys

for _p in ("/opt/trn_rl_repo", "/root/.axon_site/_ro/trn_rl_repo"):
    if _p not in sys.path:
        sys.path.append(_p)

from contextlib import ExitStack

import ml_dtypes
import numpy as np

import concourse.bass as bass
import concourse.tile as tile
from concourse import bacc, mybir
from concourse.bass_utils import run_bass_kernel_spmd

# problem shape (hardcoded per contest contract)
B, S, DM = 2, 2048, 1024
NH, NKV, DK = 16, 4, 64
HPC = NH // NKV          # 4 q heads per core
GD = HPC * DK            # 256 per-core q-proj width
N_CORES = 8
P = 128
DMC = DM // P            # 8 contraction chunks for projections
NQC = S // 512           # 4 q chunks
NKC = S // P             # 16 k chunks
NST = 2 * NQC            # 8 pipeline steps: s = 2*qc + pr
F32 = mybir.dt.float32
BF16 = mybir.dt.bfloat16
FP8 = mybir.dt.float8e4
SCALE = 1.0 / 8.0        # 1/sqrt(d_k)
EXP = mybir.ActivationFunctionType.Exp

# score/exp units per step: u = hh*NKC + kc, grouped 2 per 2-bank PSUM tile
_UNITS = 2 * NKC         # 32
_GROUPS = [2] * 16


def _body(
    ctx: ExitStack, tc: tile.TileContext, qT, kT, vT, wq, wk2, wv, wo, out, dbg=None
):
    nc = tc.nc

    const = ctx.enter_context(tc.tile_pool(name="const", bufs=1))
    QT_sb = [const.tile([P, S], BF16, name=f"QT{i}") for i in range(2)]
    KT2_sb = const.tile([P, S], BF16, name="KT2")
    V_sb = const.tile([P, NKC, DK + 1], BF16, name="Vaug")
    # W_o rows for a head PAIR stacked on 128 partitions
    WO_sb = const.tile([P, 2, DM], BF16, name="WOsb")
    WQ_sb = const.tile([P, DMC, GD], BF16, name="WQsb")
    WK2_sb = const.tile([P, DMC, P], BF16, name="WK2sb")
    WV_sb = const.tile([P, DMC, DK], BF16, name="WVsb")
    ones_sb = const.tile([P, P], F32, name="ones")

    nc.vector.memset(ones_sb[:], 1.0)
    nc.vector.memset(V_sb[:, :, DK : DK + 1], 1.0)
    nc.sync.dma_start(WK2_sb[:], wk2.rearrange("(c p) g -> p c g", p=P))
    nc.sync.dma_start(WQ_sb[:], wq.rearrange("(c p) g -> p c g", p=P))
    nc.sync.dma_start(WV_sb[:], wv.rearrange("(c p) g -> p c g", p=P))
    nc.sync.dma_start(WO_sb[:], wo.rearrange("(t p) n -> p t n", p=P))

    # PSUM budget (8 banks): psumS 2 bufs x 2 banks (score groups) +
    # psumO 2 bufs x 1 bank (PV accumulators, held across interleaved work) +
    # psum1 2 bufs x 1 bank (projections, bcast, Wo out)
    psumS = ctx.enter_context(tc.tile_pool(name="psumS", bufs=2, space="PSUM"))
    psumO = ctx.enter_context(tc.tile_pool(name="psumO", bufs=2, space="PSUM"))
    psum1 = ctx.enter_context(tc.tile_pool(name="psum1", bufs=2, space="PSUM"))

    # ---------------- projections ----------------
    xpool = ctx.enter_context(tc.tile_pool(name="xin", bufs=10))

    def load_chunks(src, pieces):
        """Allocate the 8 d_model chunks and DMA them in `pieces` column
        pieces, piece-major, so dependents can start on the first piece."""
        tiles = [xpool.tile([P, S], BF16, tag="xin", name="xc") for _ in range(DMC)]
        w = S // pieces
        for sc in range(pieces):
            for c in range(DMC):
                nc.sync.dma_start(
                    tiles[c][:, sc * w : (sc + 1) * w],
                    src[c * P : (c + 1) * P, sc * w : (sc + 1) * w],
                )
        return tiles

    kts = load_chunks(kT, 4)
    for sc in range(NQC):
        ps = psum1.tile([P, 512], F32, tag="p1")
        for c in range(DMC):
            nc.tensor.matmul(
                ps[:],
                WK2_sb[:, c, :],
                kts[c][:, sc * 512 : (sc + 1) * 512],
                start=(c == 0),
                stop=(c == DMC - 1),
            )
        nc.vector.tensor_copy(KT2_sb[:, sc * 512 : (sc + 1) * 512], ps[:])

    qts = load_chunks(qT, 4)

    def q_proj_group(ht, sc):
        ps = psum1.tile([P, 512], F32, tag="p1")
        for c in range(DMC):
            nc.tensor.matmul(
                ps[:],
                WQ_sb[:, c, ht * P : (ht + 1) * P],
                qts[c][:, sc * 512 : (sc + 1) * 512],
                start=(c == 0),
                stop=(c == DMC - 1),
            )
        nc.vector.tensor_copy(QT_sb[ht][:, sc * 512 : (sc + 1) * 512], ps[:])

    for ht in range(2):
        for sc in range(NQC):
            q_proj_group(ht, sc)

    def v_proj():
        vts = load_chunks(vT, 4)
        for sc in range(NKC):
            ps = psum1.tile([P, DK], F32, tag="p1")
            for c in range(DMC):
                nc.tensor.matmul(
                    ps[:],
                    vts[c][:, sc * P : (sc + 1) * P],
                    WV_sb[:, c, :],
                    start=(c == 0),
                    stop=(c == DMC - 1),
                )
            nc.vector.tensor_copy(V_sb[:, sc, 0:DK], ps[:])

    # ---------------- attention ----------------
    pt_pool = ctx.enter_context(tc.tile_pool(name="pt", bufs=3))
    ot_pool = ctx.enter_context(tc.tile_pool(name="ot", bufs=6))
    os_pool = ctx.enter_context(tc.tile_pool(name="os", bufs=8))
    rr_pool = ctx.enter_context(tc.tile_pool(name="rr", bufs=4))
    ob_pool = ctx.enter_context(tc.tile_pool(name="ob", bufs=3))

    # per-step state
    state = {}

    def stage1_units(s):
        """Score matmul groups + exp for step s; yields after each group."""
        qc, pr = divmod(s, 2)
        q0 = qc * 512
        PT = pt_pool.tile([P, _UNITS, 512], BF16, tag="pt", name="PT")
        state[s] = {"PT": PT}
        u0 = 0
        for gsz in _GROUPS:
            ps = psumS.tile([P, 2, 512], F32, tag="ps", name="ps")
            for j in range(gsz):
                hh, kc = divmod(u0 + j, NKC)
                d0 = hh * DK
                nc.tensor.matmul(
                    ps[:, j, :],
                    KT2_sb[d0 : d0 + DK, kc * P : (kc + 1) * P],
                    QT_sb[pr][d0 : d0 + DK, q0 : q0 + 512],
                    start=True,
                    stop=True,
                )
            nc.scalar.activation(
                PT[:, u0 : u0 + gsz, :], ps[:, 0:gsz, :], EXP, scale=SCALE
            )
            u0 += gsz
            yield

    def stage2_units(s):
        """PV chunks for step s (4 kc per unit); after the last chunk of each
        head: PSUM->SBUF copy and denominator reciprocal (both heads'
        reciprocals land in one tile, rows 64/65)."""
        st = state[s]
        PT = st["PT"]
        osbs = []
        rrs = []
        st["osbs"], st["rrs"] = osbs, rrs
        for hh in range(2):
            po = psumO.tile([DK + 1, 512], F32, tag="po", name="po")
            for kq in range(4):
                for kc in range(kq * 4, kq * 4 + 4):
                    nc.tensor.matmul(
                        po[:],
                        V_sb[:, kc, :],
                        PT[:, hh * NKC + kc, :],
                        start=(kc == 0),
                        stop=(kc == NKC - 1),
                        skip_group_check=True,
                    )
                if kq < 3:
                    yield
            osb = os_pool.tile([DK + 1, 512], BF16, tag="os", name="osb")
            nc.vector.tensor_copy(osb[:], po[:])
            rr = rr_pool.tile([DK + 1, 512], F32, tag="rr", name="rr")
            nc.vector.reciprocal(rr[DK : DK + 1, :], osb[DK : DK + 1, :])
            osbs.append(osb)
            rrs.append(rr)
            if dbg is not None and s == 0 and hh == 0:
                nc.sync.dma_start(dbg["pt"], PT[:, 0:4, :])
                pc = ob_pool.tile([DK + 1, 512], F32, tag="ob")
                nc.vector.tensor_copy(pc[:], po[:])
                nc.sync.dma_start(dbg["po"], pc[:])
            yield

    def stage3_units(s):
        """Normalize step s (one selector matmul + two scaling muls), and
        after odd steps the full output projection of the q-chunk."""
        st = state[s]
        osbs, rrs = st["osbs"], st["rrs"]
        qc, pr = divmod(s, 2)
        opair = ot_pool.tile([P, 512], BF16, tag="ot", name="opair")
        st["opair"] = opair
        for hh in range(2):
            pb = psum1.tile([DK, 512], F32, tag="p1", name="pb")
            nc.tensor.matmul(
                pb[:],
                ones_sb[DK : DK + 1, 0:DK],
                rrs[hh][DK : DK + 1, :],
                start=True,
                stop=True,
            )
            nc.vector.tensor_mul(
                opair[hh * DK : (hh + 1) * DK, :], osbs[hh][0:DK, :], pb[:]
            )
            if dbg is not None and s == 0 and hh == 0:
                nc.sync.dma_start(dbg["rr"], rrs[0][DK : DK + 1, :])
                rbdbg = ob_pool.tile([DK, 512], F32, tag="ob")
                nc.vector.tensor_copy(rbdbg[:], pb[:])
                nc.sync.dma_start(dbg["rb"], rbdbg[:])
        yield
        if pr == 1:
            q0 = qc * 512
            otn = [state[s - 1]["opair"], opair]
            for qt in range(4):
                for nch in range(2):
                    pf = psum1.tile([P, 512], F32, tag="p1", name="pf")
                    for t in range(2):
                        nc.tensor.matmul(
                            pf[:],
                            otn[t][:, qt * P : (qt + 1) * P],
                            WO_sb[:, t, nch * 512 : (nch + 1) * 512],
                            start=(t == 0),
                            stop=(t == 1),
                        )
                    ob = ob_pool.tile([P, 512], F32, tag="ob")
                    nc.vector.tensor_copy(ob[:], pf[:])
                    nc.sync.dma_start(
                        out[
                            q0 + qt * P : q0 + (qt + 1) * P,
                            nch * 512 : (nch + 1) * 512,
                        ],
                        ob[:],
                    )
                    yield
            # release prior-step state
            del state[s - 1]
            del state[s]

    def interleave(primary, *others):
        """Alternate one unit of the primary (score) stream with one unit of
        the other stages per round, so the scores — whose emission is
        throttled to the exp engine's pace by the two score PSUM buffers —
        are spread across the whole iteration and the PE queue never drains
        down to an exp-paced tail."""
        gens = [g for g in others if g is not None]
        p = primary
        oi = 0
        while p is not None or gens:
            if p is not None:
                try:
                    next(p)
                except StopIteration:
                    p = None
            if gens:
                oi %= len(gens)
                try:
                    next(gens[oi])
                    oi += 1
                except StopIteration:
                    gens.pop(oi)

    for s in range(NST + 2):
        g1 = stage1_units(s) if s < NST else None
        g2 = stage2_units(s - 1) if 1 <= s <= NST else None
        g3 = stage3_units(s - 2) if s >= 2 else None
        if s == 0:
            # V projection rides the PE while the scalar engine starts on
            # the first exp groups
            interleave(g1)
            v_proj()
        else:
            interleave(g1, g2, g3)


def _build():
    nc = bacc.Bacc(
        "TRN2",
        target_bir_lowering=False,
        debug=False,
        enable_asserts=False,
        num_devices=N_CORES,
    )
    qT = nc.dram_tensor("qT", [DM, S], BF16, kind="ExternalInput").ap()
    kT = nc.dram_tensor("kT", [DM, S], BF16, kind="ExternalInput").ap()
    vT = nc.dram_tensor("vT", [DM, S], BF16, kind="ExternalInput").ap()
    wq = nc.dram_tensor("wq", [DM, GD], BF16, kind="ExternalInput").ap()
    wk2 = nc.dram_tensor("wk2", [DM, P], BF16, kind="ExternalInput").ap()
    wv = nc.dram_tensor("wv", [DM, DK], BF16, kind="ExternalInput").ap()
    wo = nc.dram_tensor("wo", [GD, DM], BF16, kind="ExternalInput").ap()
    out = nc.dram_tensor("out", [S, DM], F32, kind="ExternalOutput").ap()

    with tile.TileContext(nc) as tc:
        with ExitStack() as ctx:
            _body(ctx, tc, qT, kT, vT, wq, wk2, wv, wo, out)
    nc.compile()
    return nc


_NC_CACHE = None


def _get_nc():
    global _NC_CACHE
    if _NC_CACHE is None:
        _NC_CACHE = _build()
    return _NC_CACHE


def _make_in_maps(q, k, v, W_q, W_k, W_v, W_o):
    bf = ml_dtypes.bfloat16
    in_maps = []
    for c in range(N_CORES):
        b, g = divmod(c, NKV)
        wk_g = np.asarray(W_k[:, g * DK : (g + 1) * DK], np.float32)
        in_maps.append(
            {
                "qT": np.ascontiguousarray(np.asarray(q[b], np.float32).T).astype(bf),
                "kT": np.ascontiguousarray(np.asarray(k[b], np.float32).T).astype(bf),
                "vT": np.ascontiguousarray(np.asarray(v[b], np.float32).T).astype(bf),
                "wq": np.ascontiguousarray(
                    np.asarray(W_q[:, g * GD : (g + 1) * GD], np.float32)
                ).astype(bf),
                "wk2": np.ascontiguousarray(
                    np.concatenate([wk_g, wk_g], axis=1)
                ).astype(bf),
                "wv": np.ascontiguousarray(
                    np.asarray(W_v[:, g * DK : (g + 1) * DK], np.float32)
                ).astype(bf),
                "wo": np.ascontiguousarray(
                    np.asarray(W_o[g * GD : (g + 1) * GD, :], np.float32)
                ).astype(bf),
            }
        )
    return in_maps


def _run(q, k, v, W_q, W_k, W_v, W_o, trace=False, **spmd_kwargs):
    nc = _get_nc()
    in_maps = _make_in_maps(q, k, v, W_q, W_k, W_v, W_o)
    res = run_bass_kernel_spmd(
        nc, in_maps, list(range(N_CORES)), trace=trace, **spmd_kwargs
    )
    out = np.zeros((B, S, DM), np.float32)
    for c in range(N_CORES):
        b, _g = divmod(c, NKV)
        out[b] += res.results[c]["out"]
    return out, res


def kernel(q, k, v, W_q, W_k, W_v, W_o):
    out, _ = _run(q, k, v, W_q, W_k, W_v, W_o)
    return out


# revision 54
# speedup vs baseline: 1.0091x; 1.0091x over previous
"""Grouped-Query Attention (B=2, S=2048, d_model=1024, 16 Q heads / 4 KV heads)
as a Trainium2 Bass/Tile kernel over 8 NeuronCores.

Sharding: core c = (batch b, kv-group g) with b = c//4, g = c%4.
Each core owns 4 consecutive Q heads (4g..4g+3) and their shared KV head g:
  - W_q column-shard [1024, 256], W_k/W_v column-shard [1024, 64],
  - W_o row-shard [256, 1024]  -> per-core partial output, host sums the
    4 group partials per batch.

Device dataflow (all matmuls contract over the SBUF partition dim):
  inputs are host-transposed to x^T [d_model, S]; projections run with the
  weight as the stationary operand:
    K^T[64,S]   = Wk^T  @ k^T   (stored twice on partitions 0-63/64-127)
    Q^T[256,S]  = Wq^T  @ q^T   (two tiles of 128 = head pairs)
    V[S,65]     = v @ Wv, plus a ones column (rowsum trick)
  attention as a 3-deep software pipeline over steps s = (q-chunk, head
  pair), with the stages' PE work INTERLEAVED in emission order so the PE
  never waits on the exp engine or the DVE reciprocal chain:
    stage 1 (step s):   S^T = K_blk @ Q^T ; P^T = exp(S^T/8)  (scalar eng)
    stage 2 (step s-1): O^T[65,q] = V_aug^T @ P^T ; copy PSUM->SBUF bf16;
                        reciprocal of denominator row (DVE)
    stage 3 (step s-2): one K=2 selector-matmul broadcasts both heads'
                        reciprocals to their partition halves; O_norm = O*r;
                        after odd steps, the head-paired output projection
"""

import sys

for _p in ("/opt/trn_rl_repo", "/root/.axon_site/_ro/trn_rl_repo"):
    if _p not in sys.path:
        sys.path.append(_p)

from contextlib import ExitStack

import ml_dtypes
import numpy as np

import concourse.bass as bass
import concourse.tile as tile
from concourse import bacc, mybir
from concourse.bass_utils import run_bass_kernel_spmd

# problem shape (hardcoded per contest contract)
B, S, DM = 2, 2048, 1024
NH, NKV, DK = 16, 4, 64
HPC = NH // NKV          # 4 q heads per core
GD = HPC * DK            # 256 per-core q-proj width
N_CORES = 8
P = 128
DMC = DM // P            # 8 contraction chunks for projections
NQC = S // 512           # 4 q chunks
NKC = S // P             # 16 k chunks
NST = 2 * NQC            # 8 pipeline steps: s = 2*qc + pr
F32 = mybir.dt.float32
BF16 = mybir.dt.bfloat16
FP8 = mybir.dt.float8e4
SCALE = 1.0 / 8.0        # 1/sqrt(d_k)
EXP = mybir.ActivationFunctionType.Exp

# score/exp units per step: u = hh*NKC + kc, grouped 2 per 2-bank PSUM tile
_UNITS = 2 * NKC         # 32
_GROUPS = [2] * 16


def _body(
    ctx: ExitStack, tc: tile.TileContext, qT, kT, vT, wq, wk2, wv, wo, out, dbg=None
):
    nc = tc.nc

    const = ctx.enter_context(tc.tile_pool(name="const", bufs=1))
    QT_sb = [const.tile([P, S], BF16, name=f"QT{i}") for i in range(2)]
    KT2_sb = const.tile([P, S], BF16, name="KT2")
    V_sb = const.tile([P, NKC, DK + 1], BF16, name="Vaug")
    # W_o rows for a head PAIR stacked on 128 partitions
    WO_sb = const.tile([P, 2, DM], BF16, name="WOsb")
    WQ_sb = const.tile([P, DMC, GD], BF16, name="WQsb")
    WK2_sb = const.tile([P, DMC, P], BF16, name="WK2sb")
    WV_sb = const.tile([P, DMC, DK], BF16, name="WVsb")
    ones_sb = const.tile([P, P], F32, name="ones")

    nc.vector.memset(ones_sb[:], 1.0)
    nc.vector.memset(V_sb[:, :, DK : DK + 1], 1.0)
    nc.sync.dma_start(WK2_sb[:], wk2.rearrange("(c p) g -> p c g", p=P))
    nc.sync.dma_start(WQ_sb[:], wq.rearrange("(c p) g -> p c g", p=P))
    nc.sync.dma_start(WV_sb[:], wv.rearrange("(c p) g -> p c g", p=P))
    nc.sync.dma_start(WO_sb[:], wo.rearrange("(t p) n -> p t n", p=P))

    # PSUM budget (8 banks): psumS 2 bufs x 2 banks (score groups) +
    # psumO 2 bufs x 1 bank (PV accumulators, held across interleaved work) +
    # psum1 2 bufs x 1 bank (projections, bcast, Wo out)
    psumS = ctx.enter_context(tc.tile_pool(name="psumS", bufs=2, space="PSUM"))
    psumO = ctx.enter_context(tc.tile_pool(name="psumO", bufs=2, space="PSUM"))
    psum1 = ctx.enter_context(tc.tile_pool(name="psum1", bufs=2, space="PSUM"))

    # ---------------- projections ----------------
    xpool = ctx.enter_context(tc.tile_pool(name="xin", bufs=10))

    def load_chunks(src, pieces):
        """Allocate the 8 d_model chunks and DMA them in `pieces` column
        pieces, piece-major, so dependents can start on the first piece."""
        tiles = [xpool.tile([P, S], BF16, tag="xin", name="xc") for _ in range(DMC)]
        w = S // pieces
        for sc in range(pieces):
            for c in range(DMC):
                nc.sync.dma_start(
                    tiles[c][:, sc * w : (sc + 1) * w],
                    src[c * P : (c + 1) * P, sc * w : (sc + 1) * w],
                )
        return tiles

    kts = load_chunks(kT, 4)
    for sc in range(NQC):
        ps = psum1.tile([P, 512], F32, tag="p1")
        for c in range(DMC):
            nc.tensor.matmul(
                ps[:],
                WK2_sb[:, c, :],
                kts[c][:, sc * 512 : (sc + 1) * 512],
                start=(c == 0),
                stop=(c == DMC - 1),
            )
        nc.vector.tensor_copy(KT2_sb[:, sc * 512 : (sc + 1) * 512], ps[:])

    qts = load_chunks(qT, 4)

    def q_proj_group(ht, sc):
        ps = psum1.tile([P, 512], F32, tag="p1")
        for c in range(DMC):
            nc.tensor.matmul(
                ps[:],
                WQ_sb[:, c, ht * P : (ht + 1) * P],
                qts[c][:, sc * 512 : (sc + 1) * 512],
                start=(c == 0),
                stop=(c == DMC - 1),
            )
        nc.vector.tensor_copy(QT_sb[ht][:, sc * 512 : (sc + 1) * 512], ps[:])

    for ht in range(2):
        for sc in range(NQC):
            q_proj_group(ht, sc)

    def v_proj():
        vts = load_chunks(vT, 4)
        for sc in range(NKC):
            ps = psum1.tile([P, DK], F32, tag="p1")
            for c in range(DMC):
                nc.tensor.matmul(
                    ps[:],
                    vts[c][:, sc * P : (sc + 1) * P],
                    WV_sb[:, c, :],
                    start=(c == 0),
                    stop=(c == DMC - 1),
                )
            nc.vector.tensor_copy(V_sb[:, sc, 0:DK], ps[:])

    # ---------------- attention ----------------
    pt_pool = ctx.enter_context(tc.tile_pool(name="pt", bufs=3))
    ot_pool = ctx.enter_context(tc.tile_pool(name="ot", bufs=6))
    os_pool = ctx.enter_context(tc.tile_pool(name="os", bufs=8))
    rr_pool = ctx.enter_context(tc.tile_pool(name="rr", bufs=4))
    ob_pool = ctx.enter_context(tc.tile_pool(name="ob", bufs=3))

    # per-step state
    state = {}

    def stage1_units(s):
        """Score matmul groups + exp for step s; yields after each group."""
        qc, pr = divmod(s, 2)
        q0 = qc * 512
        PT = pt_pool.tile([P, _UNITS, 512], BF16, tag="pt", name="PT")
        state[s] = {"PT": PT}
        u0 = 0
        for gsz in _GROUPS:
            ps = psumS.tile([P, 2, 512], F32, tag="ps", name="ps")
            for j in range(gsz):
                hh, kc = divmod(u0 + j, NKC)
                d0 = hh * DK
                nc.tensor.matmul(
                    ps[:, j, :],
                    KT2_sb[d0 : d0 + DK, kc * P : (kc + 1) * P],
                    QT_sb[pr][d0 : d0 + DK, q0 : q0 + 512],
                    start=True,
                    stop=True,
                )
            nc.scalar.activation(
                PT[:, u0 : u0 + gsz, :], ps[:, 0:gsz, :], EXP, scale=SCALE
            )
            u0 += gsz
            yield

    def stage2_units(s):
        """PV chunks for step s (4 kc per unit); after the last chunk of each
        head: PSUM->SBUF copy and denominator reciprocal (both heads'
        reciprocals land in one tile, rows 64/65)."""
        st = state[s]
        PT = st["PT"]
        osbs = []
        rrs = []
        st["osbs"], st["rrs"] = osbs, rrs
        for hh in range(2):
            po = psumO.tile([DK + 1, 512], F32, tag="po", name="po")
            for kq in range(4):
                for kc in range(kq * 4, kq * 4 + 4):
                    nc.tensor.matmul(
                        po[:],
                        V_sb[:, kc, :],
                        PT[:, hh * NKC + kc, :],
                        start=(kc == 0),
                        stop=(kc == NKC - 1),
                        skip_group_check=True,
                    )
                if kq < 3:
                    yield
            osb = os_pool.tile([DK + 1, 512], BF16, tag="os", name="osb")
            nc.vector.tensor_copy(osb[:], po[:])
            rr = rr_pool.tile([DK + 1, 512], F32, tag="rr", name="rr")
            nc.vector.reciprocal(rr[DK : DK + 1, :], osb[DK : DK + 1, :])
            osbs.append(osb)
            rrs.append(rr)
            if dbg is not None and s == 0 and hh == 0:
                nc.sync.dma_start(dbg["pt"], PT[:, 0:4, :])
                pc = ob_pool.tile([DK + 1, 512], F32, tag="ob")
                nc.vector.tensor_copy(pc[:], po[:])
                nc.sync.dma_start(dbg["po"], pc[:])
            yield

    def stage3_units(s):
        """Normalize step s (one selector matmul + two scaling muls), and
        after odd steps the full output projection of the q-chunk."""
        st = state[s]
        osbs, rrs = st["osbs"], st["rrs"]
        qc, pr = divmod(s, 2)
        opair = ot_pool.tile([P, 512], BF16, tag="ot", name="opair")
        st["opair"] = opair
        for hh in range(2):
            pb = psum1.tile([DK, 512], F32, tag="p1", name="pb")
            nc.tensor.matmul(
                pb[:],
                ones_sb[DK : DK + 1, 0:DK],
                rrs[hh][DK : DK + 1, :],
                start=True,
                stop=True,
            )
            nc.vector.tensor_mul(
                opair[hh * DK : (hh + 1) * DK, :], osbs[hh][0:DK, :], pb[:]
            )
            if dbg is not None and s == 0 and hh == 0:
                nc.sync.dma_start(dbg["rr"], rrs[0][DK : DK + 1, :])
                rbdbg = ob_pool.tile([DK, 512], F32, tag="ob")
                nc.vector.tensor_copy(rbdbg[:], pb[:])
                nc.sync.dma_start(dbg["rb"], rbdbg[:])
        yield
        if pr == 1:
            q0 = qc * 512
            otn = [state[s - 1]["opair"], opair]
            for qt in range(4):
                for nch in range(2):
                    pf = psum1.tile([P, 512], F32, tag="p1", name="pf")
                    for t in range(2):
                        nc.tensor.matmul(
                            pf[:],
                            otn[t][:, qt * P : (qt + 1) * P],
                            WO_sb[:, t, nch * 512 : (nch + 1) * 512],
                            start=(t == 0),
                            stop=(t == 1),
                        )
                    ob = ob_pool.tile([P, 512], F32, tag="ob")
                    nc.vector.tensor_copy(ob[:], pf[:])
                    nc.sync.dma_start(
                        out[
                            q0 + qt * P : q0 + (qt + 1) * P,
                            nch * 512 : (nch + 1) * 512,
                        ],
                        ob[:],
                    )
                    yield
            # release prior-step state
            del state[s - 1]
            del state[s]

    def interleave(*gens):
        """Round-robin the stage generators (a generator or a (gen, arity)
        pair taking `arity` units per round) so the PE queue alternates
        between independent streams and never stalls on the exp engine or
        the DVE reciprocal chain."""
        gens = [g if isinstance(g, tuple) else (g, 1) for g in gens if g is not None]
        while gens:
            nxt = []
            for g, ar in gens:
                alive = True
                for _ in range(ar):
                    try:
                        next(g)
                    except StopIteration:
                        alive = False
                        break
                if alive:
                    nxt.append((g, ar))
            gens = nxt

    for s in range(NST + 2):
        g1 = stage1_units(s) if s < NST else None
        g2 = stage2_units(s - 1) if 1 <= s <= NST else None
        g3 = stage3_units(s - 2) if s >= 2 else None
        if s == 0:
            # V projection rides the PE while the scalar engine starts on
            # the first exp groups
            interleave(g1)
            v_proj()
        else:
            interleave(g1, (g2, 2) if g2 is not None else None, g3)


def _build():
    nc = bacc.Bacc(
        "TRN2",
        target_bir_lowering=False,
        debug=False,
        enable_asserts=False,
        num_devices=N_CORES,
    )
    qT = nc.dram_tensor("qT", [DM, S], BF16, kind="ExternalInput").ap()
    kT = nc.dram_tensor("kT", [DM, S], BF16, kind="ExternalInput").ap()
    vT = nc.dram_tensor("vT", [DM, S], BF16, kind="ExternalInput").ap()
    wq = nc.dram_tensor("wq", [DM, GD], BF16, kind="ExternalInput").ap()
    wk2 = nc.dram_tensor("wk2", [DM, P], BF16, kind="ExternalInput").ap()
    wv = nc.dram_tensor("wv", [DM, DK], BF16, kind="ExternalInput").ap()
    wo = nc.dram_tensor("wo", [GD, DM], BF16, kind="ExternalInput").ap()
    out = nc.dram_tensor("out", [S, DM], F32, kind="ExternalOutput").ap()

    with tile.TileContext(nc) as tc:
        with ExitStack() as ctx:
            _body(ctx, tc, qT, kT, vT, wq, wk2, wv, wo, out)
    nc.compile()
    return nc


_NC_CACHE = None


def _get_nc():
    global _NC_CACHE
    if _NC_CACHE is None:
        _NC_CACHE = _build()
    return _NC_CACHE


def _make_in_maps(q, k, v, W_q, W_k, W_v, W_o):
    bf = ml_dtypes.bfloat16
    in_maps = []
    for c in range(N_CORES):
        b, g = divmod(c, NKV)
        wk_g = np.asarray(W_k[:, g * DK : (g + 1) * DK], np.float32)
        in_maps.append(
            {
                "qT": np.ascontiguousarray(np.asarray(q[b], np.float32).T).astype(bf),
                "kT": np.ascontiguousarray(np.asarray(k[b], np.float32).T).astype(bf),
                "vT": np.ascontiguousarray(np.asarray(v[b], np.float32).T).astype(bf),
                "wq": np.ascontiguousarray(
                    np.asarray(W_q[:, g * GD : (g + 1) * GD], np.float32)
                ).astype(bf),
                "wk2": np.ascontiguousarray(
                    np.concatenate([wk_g, wk_g], axis=1)
                ).astype(bf),
                "wv": np.ascontiguousarray(
                    np.asarray(W_v[:, g * DK : (g + 1) * DK], np.float32)
                ).astype(bf),
                "wo": np.ascontiguousarray(
                    np.asarray(W_o[g * GD : (g + 1) * GD, :], np.float32)
                ).astype(bf),
            }
        )
    return in_maps


def _run(q, k, v, W_q, W_k, W_v, W_o, trace=False, **spmd_kwargs):
    nc = _get_nc()
    in_maps = _make_in_maps(q, k, v, W_q, W_k, W_v, W_o)
    res = run_bass_kernel_spmd(
        nc, in_maps, list(range(N_CORES)), trace=trace, **spmd_kwargs
    )
    out = np.zeros((B, S, DM), np.float32)
    for c in range(N_CORES):
        b, _g = divmod(c, NKV)
        out[b] += res.results[c]["out"]
    return out, res


def kernel(q, k, v, W_q, W_k, W_v, W_o):
    out, _ = _run(q, k, v, W_q, W_k, W_v, W_o)
    return out
